# revision 4
# baseline (speedup 1.0000x reference)
"""ColorHistogramLoss Trainium2 kernel (8 NeuronCores, data-parallel).

Strategy: shard batch (32 -> 4 per core); each core streams its 25MB of
pixels through SBUF as 8 iterations of [128, 2048] plane-triples (4 real +
4 fake) and produces cumulative histogram-edge counts which the host
differences into the three 10-bin histograms and the scalar loss.

Counting scheme (geometric hue, v2):
- hue: hue cumulative counts are arcs of the RGB hexagon measured from
  hr=0.  With u = g-b, v = b-r, each arc boundary is a ray through the
  origin of the (u,v) plane, so every cumulative count is ONE fused test
  [sign(u) gate] * [u,v halfplane indicator] - no division, no case
  masks.  Antipodal ray pairs (e, e+3) share the same boundary line, so
  8 of the 9 edges pack pairwise into 4 custom-DVE ops (2 counts per op
  via the cnt + 4096*cnt accumulator packing); the 9th (hr<3 == [u>=0])
  is a Sign activation on ScalarE.
- sat: sat < c  <=>  c*mx > d, with d = max(|u|,|v|,|u+v|) computed in
  ONE custom op (max(a,b,c) = max(max3, -min3) fits the 8-stage DVE
  budget); 9 edges = 4 dual-edge customs + the 0.7 edge packed with
  val-0.7 in a mixed custom (SVP2).
- val: mx < c; the 8 remaining edges are Sign(mx-e) activations with
  fused accumulation on ScalarE (host decodes N_lt = (N - sum_sign)/2),
  batched over iteration PAIRS ([P,2,2048] access pattern) to amortize
  the ~0.5us fixed cost per activation; the final iteration runs them
  as dual-edge customs on VectorE so ScalarE is not the pipeline tail.

All counts are exact integer arithmetic in f32; only boundary-ulp
pixels differ from the f32 reference (rel err ~4e-5).
"""

import sys

if "/opt/trn_rl_repo" not in sys.path:
    sys.path.insert(0, "/opt/trn_rl_repo")

import numpy as np

from concourse import bacc, mybir, tile
from concourse import bass_utils

# ---- problem constants (hardcoded; kernel.py must be self-contained) ----
B, C, H, W = 32, 3, 512, 512
NCORES = 8
BPC = B // NCORES            # batches per core
P, F = 128, 2048             # SBUF tile: one [512,512] plane = [128, 2048]
NITER = 2 * BPC              # 4 real + 4 fake plane-triple iterations
ACCW = 20                    # accumulator columns per iteration
NPIX = B * H * W             # pixels per full histogram
ALPHA, BETA, GAMMA = 0.3, 0.4, 0.4

AF = mybir.AluOpType
F32 = mybir.dt.float32

LAST_EXEC_NS = None
_CACHE = {}

PACK = 4096.0  # dual-count packing: accum = cntA + PACK*cntB (exact in f32)
C23 = float(np.float32(2.0) / np.float32(3.0))

# val cumulative edges counted on ScalarE via Sign (0.7 rides SVP2 on V)
SVAL_EDGES = (0.1, 0.2, 0.3, 0.4, 0.5, 0.6, 0.8, 0.9)


def _register_custom_ops():
    """Author + register fused DVE ops in the dve_ops registry at runtime
    (the repo list is read-only; registration is by-name so appending to the
    module-level OPS list is sufficient for table-gen and tracing)."""
    from concourse import dve_ops
    from concourse.dve_spec import (
        C0, C1, C2, Spec, Src0, Src1, Zero, One, _has_src1, lower, maxx,
        minn, select,
    )
    from concourse.dve_uop import DveOpSpec

    if hasattr(dve_ops, "HPA"):
        return dve_ops

    from operator import add as _add

    def _accref(body_fn):
        def ref(in0, in1, c0, c1, c2):
            b = body_fn(
                np.asarray(in0, np.float32),
                np.asarray(in1, np.float32) if in1 is not None else None,
                c0, c1, c2,
            ).astype(np.float32)
            return b, b.reshape(b.shape[0], -1).sum(axis=-1, keepdims=True)
        return ref

    # gate: 1 where u>=0 else PACK (routes the count into the high field)
    gate = select(Src0 >= Zero, One, C1)

    defs = [
        # hue pair, A-form: t = (Src0 + C0*Src1 <= 0); accum t*(1|C1 by sign)
        (
            "HPA",
            Spec(
                body=((Src0 + C0 * Src1) <= Zero) * gate,
                accum=_add,
                accum_init=Zero,
                reference=_accref(
                    lambda u, v, c0, c1, c2: ((u + np.float32(c0) * v) <= 0)
                    * np.where(u >= 0, 1.0, c1)
                ),
            ),
        ),
        # hue pair, B-form: t = (Src1 + C0*Src0 <= 0)
        (
            "HPB",
            Spec(
                body=((Src1 + C0 * Src0) <= Zero) * gate,
                accum=_add,
                accum_init=Zero,
                reference=_accref(
                    lambda u, v, c0, c1, c2: ((v + np.float32(c0) * u) <= 0)
                    * np.where(u >= 0, 1.0, c1)
                ),
            ),
        ),
        # t3 = max(0, v, u+v); mx = r + t3 (since u+v = g-r, v = b-r)
        (
            "M3",
            Spec(
                body=maxx(maxx(Src0 + Src1, Src1), Zero),
                reference=lambda in0, in1, c0, c1, c2: np.maximum(
                    np.maximum(in0 + in1, in1), 0.0
                ).astype(np.float32),
            ),
        ),
        # d = max(|u|, |v|, |u+v|) = max(max(u,v,u+v), -min(u,v,u+v))
        (
            "D3",
            Spec(
                body=maxx(
                    maxx(maxx(Src0, Src1), Src0 + Src1),
                    Zero - minn(minn(Src0, Src1), Src0 + Src1),
                ),
                reference=lambda in0, in1, c0, c1, c2: np.maximum(
                    np.maximum(np.abs(in0), np.abs(in1)), np.abs(in0 + in1)
                ).astype(np.float32),
            ),
        ),
        # dual sat-edge count: accum = #{in0*c0 > in1} + c1*#{in0*c2 > in1}
        (
            "EDGE2D",
            Spec(
                body=(Src0 * C0 > Src1) + C1 * ((Src0 * C2) > Src1),
                accum=_add,
                accum_init=Zero,
                reference=_accref(
                    lambda a, b, c0, c1, c2: (
                        (a * np.float32(c0) > b) + c1 * ((a * np.float32(c2)) > b)
                    )
                ),
            ),
        ),
        # mixed: accum = #{in0*c0 > in1} + c1*#{in0 < c0}  (sat-c0 + val-c0)
        (
            "SVP2",
            Spec(
                body=(Src0 * C0 > Src1) + C1 * (Src0 < C0),
                accum=_add,
                accum_init=Zero,
                reference=_accref(
                    lambda a, b, c0, c1, c2: (
                        (a * np.float32(c0) > b) + c1 * (a < np.float32(c0))
                    )
                ),
            ),
        ),
        # dual val-edge count: accum = #{in0 < c0} + c1*#{in0 < c2}
        (
            "EDGE2",
            Spec(
                body=(Src0 < C0) + C1 * (Src0 < C2),
                accum=_add,
                accum_init=Zero,
                reference=_accref(
                    lambda a, b, c0, c1, c2: (
                        (a < np.float32(c0)) + c1 * (a < np.float32(c2))
                    )
                ),
            ),
        ),
    ]
    for name, spec in defs:
        row = 1 + len(dve_ops.OPS)
        shas = {}
        for ver in ("v3", "v4"):
            uops = lower(spec, ver=ver)
            shas[ver] = DveOpSpec(
                name=name, opcode=row, uops=uops, rd1_en=_has_src1(spec)
            ).sha(ver)
        op = dve_ops.DveOp(name, spec, False, uops_sha=shas)
        dve_ops.OPS.append(op)
        dve_ops.CUSTOM_DVE_SPECS[name] = spec
        dve_ops._SUB_OPCODE_FOR_NAME[name] = row
        setattr(dve_ops, name, op)
    return dve_ops


def _build():
    dve_ops = _register_custom_ops()
    nc = bacc.Bacc(
        "TRN2", target_bir_lowering=False, debug=False, num_devices=NCORES
    )
    xr = nc.dram_tensor("x_real", [BPC * C * P, F], F32, kind="ExternalInput").ap()
    xf = nc.dram_tensor("x_fake", [BPC * C * P, F], F32, kind="ExternalInput").ap()
    out = nc.dram_tensor("out", [NITER * P, ACCW], F32, kind="ExternalOutput").ap()

    with tile.TileContext(nc) as tc:
        with tc.tile_pool(name="main", bufs=2) as io_pool, tc.tile_pool(
            name="tmp", bufs=1
        ) as tmp_pool:
            # per-edge bias tiles for ScalarE Sign activations (bias = -edge)
            sbias = []
            for e in SVAL_EDGES + (0.0,):       # 0.0: E-count sign(u)
                bt = tmp_pool.tile([P, 1], F32, tag=f"sb{int(e*10)}",
                                   name=f"sb{int(e*10)}")
                nc.gpsimd.memset(bt[:], -e)
                sbias.append(bt)
            SIGN = mybir.ActivationFunctionType.Sign
            scr2 = tmp_pool.tile([P, F], F32, tag="scr2", name="scr2")
            scr2b = tmp_pool.tile([P, 2, F], F32, tag="scr2b", name="scr2b")
            for it in range(NITER):
                src = xr if it < BPC else xf
                bi = it % BPC

                def plane(c):
                    q = bi * C + c
                    return src[q * P : (q + 1) * P, :]

                r = io_pool.tile([P, F], F32, tag="r")
                g = io_pool.tile([P, F], F32, tag="g")
                bl = io_pool.tile([P, F], F32, tag="bl")
                # g, bl first: Pool's opening op (u = g - bl) needs them
                nc.sync.dma_start(g[:], plane(1))
                nc.sync.dma_start(bl[:], plane(2))
                nc.sync.dma_start(r[:], plane(0))

                # u and mx live in [P, 2, F] pair tiles so ScalarE can run
                # one batched activation over two iterations' data
                if it % 2 == 0:
                    upair = io_pool.tile([P, 2, F], F32, tag="upair")
                    mxpair = io_pool.tile([P, 2, F], F32, tag="mxpair")
                u = upair[:, it % 2, :]
                mx = mxpair[:, it % 2, :]
                v = io_pool.tile([P, F], F32, tag="v")
                t3 = tmp_pool.tile([P, F], F32, tag="t3", name="t3")
                d = io_pool.tile([P, F], F32, tag="d")
                acc = io_pool.tile([P, ACCW], F32, tag="acc")
                scr = tmp_pool.tile([P, F], F32, tag="scr", name="scr")

                V = nc.vector
                # u, v on the Pool engine (V is the bottleneck; Pool is idle)
                nc.gpsimd.tensor_tensor(u, g[:], bl[:], AF.subtract)
                nc.gpsimd.tensor_tensor(v[:], bl[:], r[:], AF.subtract)
                # hue pairs on (u, v): acc0..3
                V._custom_dve(dve_ops.HPA, out=scr[:], in0=u, in1=v[:],
                              s0=0.6, s1=PACK, accum_out=acc[:, 0:1])
                V._custom_dve(dve_ops.HPB, out=scr[:], in0=u, in1=v[:],
                              s0=0.8, s1=PACK, accum_out=acc[:, 1:2])
                V._custom_dve(dve_ops.HPB, out=scr[:], in0=u, in1=v[:],
                              s0=0.2, s1=PACK, accum_out=acc[:, 2:3])
                V._custom_dve(dve_ops.HPB, out=scr[:], in0=u, in1=v[:],
                              s0=-C23, s1=PACK, accum_out=acc[:, 3:4])
                # d = max(|u|,|v|,|u+v|)
                V._custom_dve(dve_ops.D3, out=d[:], in0=u, in1=v[:])
                # mx = r + max(0, v, u+v): V touches only the r raw plane,
                # keeping V off the g/bl tiles the Pool engine is reading
                V._custom_dve(dve_ops.M3, out=t3[:], in0=u, in1=v[:])
                V.tensor_tensor(mx, t3[:], r[:], AF.add)
                # sat pairs on (mx, d): acc5..8
                for j, (c1, c2) in enumerate(
                    ((0.1, 0.2), (0.3, 0.6), (0.4, 0.8), (0.9, 0.5))
                ):
                    V._custom_dve(dve_ops.EDGE2D, out=scr[:], in0=mx,
                                  in1=d[:], s0=c1, s1=PACK, imm2=c2,
                                  accum_out=acc[:, 5 + j : 6 + j])
                # sat-0.7 + val-0.7: acc9
                V._custom_dve(dve_ops.SVP2, out=scr[:], in0=mx, in1=d[:],
                              s0=0.7, s1=PACK, accum_out=acc[:, 9:10])

                if it in (1, 3, 5):
                    # ScalarE, batched over the iteration pair: val-8 + E
                    # acc10..17 (val sign-sums over 2*F), acc4 (E sign-sum)
                    for k in range(8):
                        nc.scalar.activation(
                            scr2b[:], mxpair[:], SIGN, bias=sbias[k][:],
                            accum_out=acc[:, 10 + k : 11 + k],
                        )
                    nc.scalar.activation(
                        scr2b[:], upair[:], SIGN, bias=sbias[8][:],
                        accum_out=acc[:, 4:5],
                    )
                elif it == 6:
                    # solo activations (iter 7 pairs with nothing on S)
                    for k in range(8):
                        nc.scalar.activation(
                            scr2[:], mx, SIGN, bias=sbias[k][:],
                            accum_out=acc[:, 10 + k : 11 + k],
                        )
                    nc.scalar.activation(
                        scr2[:], u, SIGN, bias=sbias[8][:],
                        accum_out=acc[:, 4:5],
                    )
                elif it == 7:
                    # final iteration on VectorE so ScalarE isn't the tail:
                    # acc10..13 dual val counts, acc4 E count (direct)
                    for j, (c1, c2) in enumerate(
                        ((0.1, 0.2), (0.3, 0.4), (0.5, 0.6), (0.8, 0.9))
                    ):
                        V._custom_dve(dve_ops.EDGE2, out=scr[:], in0=mx,
                                      s0=c1, s1=PACK, imm2=c2,
                                      accum_out=acc[:, 10 + j : 11 + j])
                    V.tensor_scalar(scr[:], u, 0.0, None, AF.is_ge, AF.add,
                                    accum_out=acc[:, 4:5])
                nc.sync.dma_start(out[it * P : (it + 1) * P, :], acc[:, :])

    nc.compile()
    return nc


def _register_ntff_hook():
    """Register the axon NTFF profiling hook (the container's antenv stub
    lacks axon_hooks, so trn_boot's registration was skipped). Also keep
    profile artifacts local instead of uploading to a share."""
    import types

    import antenv

    if "antenv.axon_hooks" not in sys.modules:
        mod = types.ModuleType("antenv.axon_hooks")
        holder = [None]
        mod.set_axon_ntff_profile_hook = lambda h: holder.__setitem__(0, h)
        mod.get_axon_ntff_profile_hook = lambda: holder[0]
        sys.modules["antenv.axon_hooks"] = mod
        antenv.axon_hooks = mod
    from antenv import axon_hooks

    if axon_hooks.get_axon_ntff_profile_hook() is None:
        from trn_agent_boot.trn_boot import _ntff_profile_via_ctypes

        axon_hooks.set_axon_ntff_profile_hook(
            _ntff_profile_via_ctypes("/opt/axon/libaxon_pjrt.so")
        )
    bass_utils.upload_artifacts = lambda tmpdir: tmpdir


def _get_nc():
    if "nc" not in _CACHE:
        _CACHE["nc"] = _build()
    return _CACHE["nc"]


def kernel(x_real: np.ndarray, x_fake: np.ndarray) -> np.ndarray:
    global LAST_EXEC_NS
    nc = _get_nc()

    in_maps = []
    for c in range(NCORES):
        sl = slice(c * BPC, (c + 1) * BPC)
        in_maps.append(
            {
                "x_real": np.ascontiguousarray(x_real[sl]).reshape(BPC * C * P, F),
                "x_fake": np.ascontiguousarray(x_fake[sl]).reshape(BPC * C * P, F),
            }
        )

    import os

    trace = bool(int(os.environ.get("KERNEL_TRACE", "0")))
    if trace:
        _register_ntff_hook()
    res = bass_utils.run_bass_kernel_spmd(
        nc, in_maps, core_ids=list(range(NCORES)), trace=trace
    )
    LAST_EXEC_NS = res.exec_time_ns
    _CACHE["last_res"] = res

    # Decode.  Packed count cols are exact ints in f32: split via % and //.
    # Sign-sum cols decode as N_lt = (N_cover - S)/2.
    PK = int(PACK)
    hue_lo = np.zeros((2, 4))      # A,B,C,D
    hue_hi = np.zeros((2, 4))      # F,G,H,I tilde counts (u<0 side)
    E_sign = np.zeros(2)           # sign-sum for u (iters 0..6)
    E_direct = 0.0                 # iter-7 direct count
    sat_lo = np.zeros((2, 5))      # C_sat(0.1), (0.3), (0.4), (0.9), (0.7)
    sat_hi = np.zeros((2, 4))      # C_sat(0.2), (0.6), (0.8), (0.5)
    val07 = np.zeros(2)
    sign_sums = np.zeros((2, 8))
    val_direct = np.zeros(8)       # iter-7 val counts (EDGE2, SVAL order)
    for core_out in res.results:
        o = np.asarray(core_out["out"]).reshape(NITER, P, ACCW).astype(np.int64)
        for t, sl in ((0, slice(0, BPC)), (1, slice(BPC, NITER))):
            blk = o[sl]
            packed = blk[:, :, 0:4]
            hue_lo[t] += (packed % PK).sum(axis=(0, 1))
            hue_hi[t] += (packed // PK).sum(axis=(0, 1))
            sp = blk[:, :, 5:9]
            sat_lo[t, :4] += (sp % PK).sum(axis=(0, 1))
            sat_hi[t] += (sp // PK).sum(axis=(0, 1))
            sat_lo[t, 4] += (blk[:, :, 9] % PK).sum()
            val07[t] += (blk[:, :, 9] // PK).sum()
            if t == 0:
                # batched pair sign-sums live in iters 1, 3 only (cover 0-3)
                sign_sums[0] += blk[(1, 3), :, 10:18].sum(axis=(0, 1))
                E_sign[0] += blk[(1, 3), :, 4].sum()
            else:
                # iter 5 covers (4,5); iter 6 solo; iter 7 direct counts
                sign_sums[1] += blk[(1, 2), :, 10:18].sum(axis=(0, 1))
                E_sign[1] += blk[(1, 2), :, 4].sum()
                vp = blk[-1, :, 10:14]
                val_direct[0:8:2] += (vp % PK).sum(axis=0)
                val_direct[1:8:2] += (vp // PK).sum(axis=0)
                E_direct += blk[-1, :, 4].sum()

    # hue cumulative counts [2, 9]
    # E = count[u >= 0] = (N_cover + sign_sum)/2 for sign(u - 0)
    E = np.array([NPIX, NPIX * 3 / 4]) / 2.0 + E_sign / 2.0
    E[1] += E_direct
    C_hue = np.stack([
        hue_lo[:, 0], hue_lo[:, 1], hue_lo[:, 2], hue_lo[:, 3], E,
        NPIX - hue_hi[:, 0], NPIX - hue_hi[:, 1],
        NPIX - hue_hi[:, 2], NPIX - hue_hi[:, 3],
    ], axis=1)
    C_sat = np.stack([
        sat_lo[:, 0], sat_hi[:, 0], sat_lo[:, 1], sat_lo[:, 2], sat_hi[:, 3],
        sat_hi[:, 1], sat_lo[:, 4], sat_hi[:, 2], sat_lo[:, 3],
    ], axis=1)
    # val: signs cover all real iters, 3/4 of fake; iter-7 counted directly
    N_signs = np.array([NPIX, NPIX * 3 / 4])
    v8 = (N_signs[:, None] - sign_sums) / 2.0
    v8[1] += val_direct
    C_val = np.stack([
        v8[:, 0], v8[:, 1], v8[:, 2], v8[:, 3], v8[:, 4], v8[:, 5],
        val07, v8[:, 6], v8[:, 7],
    ], axis=1)

    loss = 0.0
    for wgt, Cc in ((ALPHA, C_hue), (BETA, C_sat), (GAMMA, C_val)):
        hist = np.zeros((2, 10))
        hist[:, 0] = Cc[:, 0]
        hist[:, 1:9] = Cc[:, 1:] - Cc[:, :-1]
        hist[:, 9] = NPIX - Cc[:, 8]
        loss += wgt * np.abs(hist[0] - hist[1]).mean()
    return np.asarray(loss, dtype=np.float32)



# revision 15
# speedup vs baseline: 1.1932x; 1.1932x over previous
"""ColorHistogramLoss Trainium2 kernel (8 NeuronCores, data-parallel).

Strategy: shard batch (32 -> 4 per core); each core streams its 25MB of
pixels through SBUF as 8 iterations of [128, 2048] plane-triples (4 real +
4 fake).  Counting is split across four engines so the DVE (the old
bottleneck) only carries what no other engine can:

- hue (9 edges): geometric ray tests on (u=g-b, v=b-r) as 4 dual-packed
  f32 custom-DVE ops (antipodal ray pairs share a boundary line; the
  sign(u) gate routes counts into a cnt + 4096*cnt packed accumulator)
  plus an E = #[u>=0] sign-sum on ScalarE.  f32 because the custom-DVE
  uop pipeline miscomputes on fp16 operands (measured).
- sat (9 edges): sat < c  <=>  mn/mx > 1-c.  ScalarE computes
  rmx = reciprocal(mx16); VectorE forms ratio16 = mn16 * rmx and then
  nine fp16 tensor_scalar indicator tiles (no accum -> 4x DVE mode,
  ~0.6us each); the TENSOR engine colsums each indicator via one-hot
  stationaries into a PSUM accumulator (216ns per 512-chunk, running
  concurrently with the DVE at no cost), accumulated over all 8
  iterations; one DVE reduce at the end reads the 18 totals.
- val (9 edges): Sign activations on fp16 mx pairs on ScalarE with fused
  accumulation (host decodes N_lt = (N - sign_sum)/2), batched over
  iteration pairs to amortize the fixed activation cost.
- min/max chains run in fp16 on the DVE at 2x (r16/g16/b16 casts ride
  the ScalarE Copy activation; Sign/Reciprocal/Copy share one act table
  so there is a single table load).

All counts are exact integers; only boundary-ulp pixels (fp16 rounding
of mx/mn/ratio) differ from the f32 reference (rel err ~6e-3 measured
against the reference loss, gate is 2e-2).
"""

import sys

if "/opt/trn_rl_repo" not in sys.path:
    sys.path.insert(0, "/opt/trn_rl_repo")

import numpy as np

from concourse import bacc, mybir, tile
from concourse import bass_utils

# ---- problem constants (hardcoded; kernel.py must be self-contained) ----
B, C, H, W = 32, 3, 512, 512
NCORES = 8
BPC = B // NCORES            # batches per core
P, F = 128, 2048             # SBUF tile: one [512,512] plane = [128, 2048]
NITER = 2 * BPC              # 4 real + 4 fake plane-triple iterations
ACCW = 20                    # accumulator columns per iteration
NPIX = B * H * W             # pixels per full histogram
ALPHA, BETA, GAMMA = 0.3, 0.4, 0.4

AF = mybir.AluOpType
F32 = mybir.dt.float32
F16 = mybir.dt.float16

LAST_EXEC_NS = None
_CACHE = {}

PACK = 4096.0  # dual-count packing: accum = cntA + PACK*cntB (exact in f32)
C23 = float(np.float32(2.0) / np.float32(3.0))

SVAL_EDGES = (0.1, 0.2, 0.3, 0.4, 0.5, 0.6, 0.7, 0.8, 0.9)
SAT_EDGES = (0.1, 0.2, 0.3, 0.4, 0.5, 0.6, 0.7, 0.8, 0.9)
NSAT = len(SAT_EDGES)


def _register_custom_ops():
    """Author + register fused DVE ops in the dve_ops registry at runtime
    (the repo list is read-only; registration is by-name so appending to the
    module-level OPS list is sufficient for table-gen and tracing)."""
    from concourse import dve_ops
    from concourse.dve_spec import (
        C0, C1, C2, Spec, Src0, Src1, Zero, One, _has_src1, lower, maxx,
        minn, select,
    )
    from concourse.dve_uop import DveOpSpec

    if hasattr(dve_ops, "HPA"):
        return dve_ops

    from operator import add as _add

    def _accref(body_fn):
        def ref(in0, in1, c0, c1, c2):
            b = body_fn(
                np.asarray(in0, np.float32),
                np.asarray(in1, np.float32) if in1 is not None else None,
                c0, c1, c2,
            ).astype(np.float32)
            return b, b.reshape(b.shape[0], -1).sum(axis=-1, keepdims=True)
        return ref

    # gate: 1 where u>=0 else PACK (routes the count into the high field)
    gate = select(Src0 >= Zero, One, C1)

    defs = [
        # hue pair, A-form: t = (Src0 + C0*Src1 <= 0); accum t*(1|C1 by sign)
        (
            "HPA",
            Spec(
                body=((Src0 + C0 * Src1) <= Zero) * gate,
                accum=_add,
                accum_init=Zero,
                reference=_accref(
                    lambda u, v, c0, c1, c2: ((u + np.float32(c0) * v) <= 0)
                    * np.where(u >= 0, 1.0, c1)
                ),
            ),
        ),
        # hue pair, B-form: t = (Src1 + C0*Src0 <= 0)
        (
            "HPB",
            Spec(
                body=((Src1 + C0 * Src0) <= Zero) * gate,
                accum=_add,
                accum_init=Zero,
                reference=_accref(
                    lambda u, v, c0, c1, c2: ((v + np.float32(c0) * u) <= 0)
                    * np.where(u >= 0, 1.0, c1)
                ),
            ),
        ),
    ]
    for name, spec in defs:
        row = 1 + len(dve_ops.OPS)
        shas = {}
        for ver in ("v3", "v4"):
            uops = lower(spec, ver=ver)
            shas[ver] = DveOpSpec(
                name=name, opcode=row, uops=uops, rd1_en=_has_src1(spec)
            ).sha(ver)
        op = dve_ops.DveOp(name, spec, False, uops_sha=shas)
        dve_ops.OPS.append(op)
        dve_ops.CUSTOM_DVE_SPECS[name] = spec
        dve_ops._SUB_OPCODE_FOR_NAME[name] = row
        setattr(dve_ops, name, op)
    return dve_ops


def _build():
    dve_ops = _register_custom_ops()
    nc = bacc.Bacc(
        "TRN2", target_bir_lowering=False, debug=False, num_devices=NCORES
    )
    xr = nc.dram_tensor("x_real", [BPC * C * P, F], F32, kind="ExternalInput").ap()
    xf = nc.dram_tensor("x_fake", [BPC * C * P, F], F32, kind="ExternalInput").ap()
    out = nc.dram_tensor("out", [NITER * P, ACCW], F32, kind="ExternalOutput").ap()
    sat_out = nc.dram_tensor("sat", [P, 2], F32, kind="ExternalOutput").ap()

    SIGN = mybir.ActivationFunctionType.Sign
    COPY = mybir.ActivationFunctionType.Copy
    LN = mybir.ActivationFunctionType.Ln

    with tile.TileContext(nc) as tc:
        with tc.tile_pool(name="main", bufs=2) as io_pool, tc.tile_pool(
            name="tmp", bufs=1
        ) as tmp_pool, tc.tile_pool(name="ps", bufs=1, space="PSUM") as ppool:
            # per-edge bias tiles for ScalarE Sign activations (bias = -edge)
            sbias = []
            for e in SVAL_EDGES + (0.0,):       # 0.0: E-count sign(u)
                bt = tmp_pool.tile([P, 1], F32, tag=f"sb{int(e*10)}",
                                   name=f"sb{int(e*10)}")
                nc.gpsimd.memset(bt[:], -e)
                sbias.append(bt)
            # one-hot stationaries for the PE colsum (col j = ones)
            stats = []
            for j in range(NSAT):
                st = tmp_pool.tile([P, NSAT], F16, tag=f"st{j}", name=f"st{j}")
                nc.vector.memset(st[:], 0.0)
                nc.vector.memset(st[:, j : j + 1], 1.0)
                stats.append(st)
            psum = ppool.tile([P, F], F32, tag="psum")
            scr2b = tmp_pool.tile([P, 2, F], F16, tag="scr2b", name="scr2b")
            ind_rot = [
                tmp_pool.tile([P, F], F16, tag=f"ind{k}", name=f"ind{k}")
                for k in range(2)
            ]
            acc_sat = tmp_pool.tile([P, 2], F32, tag="acc_sat", name="acc_sat")
            nc.vector.memset(acc_sat[:], 0.0)
            # f32 scratch shared by the custom-DVE dummy outs and the final
            # PSUM readback
            scr = tmp_pool.tile([P, F], F32, tag="scr", name="scr")

            V = nc.vector
            S = nc.scalar
            for it in range(NITER):
                src = xr if it < BPC else xf
                bi = it % BPC

                def plane(c):
                    q = bi * C + c
                    return src[q * P : (q + 1) * P, :]

                r = io_pool.tile([P, F], F32, tag="r")
                g = io_pool.tile([P, F], F32, tag="g")
                bl = io_pool.tile([P, F], F32, tag="bl")
                nc.sync.dma_start(g[:], plane(1))
                nc.sync.dma_start(bl[:], plane(2))
                nc.sync.dma_start(r[:], plane(0))

                # u and mx16 live in [P, 2, F] pair tiles so ScalarE can run
                # one batched activation over two iterations' data
                if it % 2 == 0:
                    upair = io_pool.tile([P, 2, F], F32, tag="upair")
                    mxpair = io_pool.tile([P, 2, F], F16, tag="mxpair")
                u = upair[:, it % 2, :]
                mx16 = mxpair[:, it % 2, :]
                v = tmp_pool.tile([P, F], F32, tag="v", name="v")
                r16 = io_pool.tile([P, F], F16, tag="r16")
                g16 = io_pool.tile([P, F], F16, tag="g16")
                b16 = io_pool.tile([P, F], F16, tag="b16")
                m1 = tmp_pool.tile([P, F], F16, tag="m1", name="m1")
                mn1 = tmp_pool.tile([P, F], F16, tag="mn1", name="mn1")
                mn16 = tmp_pool.tile([P, F], F16, tag="mn16", name="mn16")
                rmx = io_pool.tile([P, F], F16, tag="rmx")
                ratio = io_pool.tile([P, F], F16, tag="ratio")
                acc = io_pool.tile([P, ACCW], F32, tag="acc")

                # ScalarE: fp16 casts of the three planes
                S.activation(g16[:], g[:], COPY)
                S.activation(b16[:], bl[:], COPY)
                S.activation(r16[:], r[:], COPY)

                # DVE: u, v in f32 (custom-DVE hue ops need f32 operands)
                V.tensor_tensor(u, g[:], bl[:], AF.subtract)
                V.tensor_tensor(v[:], bl[:], r[:], AF.subtract)
                # hue pairs on (u, v): acc0..3
                V._custom_dve(dve_ops.HPA, out=scr[:], in0=u, in1=v[:],
                              s0=0.6, s1=PACK, accum_out=acc[:, 0:1])
                V._custom_dve(dve_ops.HPB, out=scr[:], in0=u, in1=v[:],
                              s0=0.8, s1=PACK, accum_out=acc[:, 1:2])
                V._custom_dve(dve_ops.HPB, out=scr[:], in0=u, in1=v[:],
                              s0=0.2, s1=PACK, accum_out=acc[:, 2:3])
                V._custom_dve(dve_ops.HPB, out=scr[:], in0=u, in1=v[:],
                              s0=-C23, s1=PACK, accum_out=acc[:, 3:4])
                # fp16 min/max chains (2x DVE mode)
                V.tensor_tensor(m1[:], r16[:], g16[:], AF.max)
                V.tensor_tensor(mx16, m1[:], b16[:], AF.max)
                V.tensor_tensor(mn1[:], r16[:], g16[:], AF.min)
                V.tensor_tensor(mn16[:], mn1[:], b16[:], AF.min)
                # sat < c  <=>  mn/mx > 1-c  <=>  ln(mn) - ln(mx) > ln(1-c).
                # (ScalarE Reciprocal is blocked for accuracy; Ln shares the
                # natural_log act table with Sign and Copy -> no reloads.
                # ln(0) = -inf keeps the mn==0 pixels in the right bin.)
                S.activation(rmx[:], mx16, LN)
                S.activation(ratio[:], mn16[:], LN)
                w = tmp_pool.tile([P, F], F16, tag="w", name="w")
                V.tensor_tensor(w[:], ratio[:], rmx[:], AF.subtract)
                # sat indicators (4x DVE mode, no accum) + PE colsum into PSUM
                base = 0 if it < BPC else 32
                rows = slice(base, base + NSAT)
                for j, c in enumerate(SAT_EDGES):
                    indt = ind_rot[j % 2][:]
                    V.tensor_scalar(indt, w[:], float(np.log(1.0 - c)), None,
                                    AF.is_gt)
                    for cj in range(4):
                        nc.tensor.matmul(
                            psum[rows, cj * 512 : (cj + 1) * 512],
                            stats[j][:, :],
                            indt[:, cj * 512 : (cj + 1) * 512],
                            start=(it % BPC == 0 and j == 0),
                            stop=(it % BPC == BPC - 1 and j == NSAT - 1),
                        )

                if it % 2 == 1:
                    # ScalarE, batched over the iteration pair: val-9 + E
                    # acc10..18 (val sign-sums over 2*F on fp16 mx), acc4 (E)
                    for k in range(9):
                        S.activation(
                            scr2b[:], mxpair[:], SIGN, bias=sbias[k][:],
                            accum_out=acc[:, 10 + k : 11 + k],
                        )
                    S.activation(
                        scr2b[:], upair[:], SIGN,
                        bias=sbias[9][:], accum_out=acc[:, 4:5],
                    )
                nc.sync.dma_start(out[it * P : (it + 1) * P, :], acc[:, :])

            # final: read the PE sat accumulators out of PSUM
            V.tensor_scalar(scr[0:NSAT, :], psum[0:NSAT, :], 1.0, None,
                            AF.mult, AF.add, accum_out=acc_sat[0:NSAT, 0:1])
            V.tensor_scalar(scr[0:NSAT, :], psum[32 : 32 + NSAT, :], 1.0,
                            None, AF.mult, AF.add,
                            accum_out=acc_sat[32 : 32 + NSAT, 1:2])
            nc.sync.dma_start(sat_out, acc_sat[:])

    nc.compile()
    return nc


def _register_ntff_hook():
    """Register the axon NTFF profiling hook (the container's antenv stub
    lacks axon_hooks, so trn_boot's registration was skipped). Also keep
    profile artifacts local instead of uploading to a share."""
    import types

    import antenv

    if "antenv.axon_hooks" not in sys.modules:
        mod = types.ModuleType("antenv.axon_hooks")
        holder = [None]
        mod.set_axon_ntff_profile_hook = lambda h: holder.__setitem__(0, h)
        mod.get_axon_ntff_profile_hook = lambda: holder[0]
        sys.modules["antenv.axon_hooks"] = mod
        antenv.axon_hooks = mod
    from antenv import axon_hooks

    if axon_hooks.get_axon_ntff_profile_hook() is None:
        from trn_agent_boot.trn_boot import _ntff_profile_via_ctypes

        axon_hooks.set_axon_ntff_profile_hook(
            _ntff_profile_via_ctypes("/opt/axon/libaxon_pjrt.so")
        )
    bass_utils.upload_artifacts = lambda tmpdir: tmpdir


def _get_nc():
    if "nc" not in _CACHE:
        _CACHE["nc"] = _build()
    return _CACHE["nc"]


def kernel(x_real: np.ndarray, x_fake: np.ndarray) -> np.ndarray:
    global LAST_EXEC_NS
    nc = _get_nc()

    in_maps = []
    for c in range(NCORES):
        sl = slice(c * BPC, (c + 1) * BPC)
        in_maps.append(
            {
                "x_real": np.ascontiguousarray(x_real[sl]).reshape(BPC * C * P, F),
                "x_fake": np.ascontiguousarray(x_fake[sl]).reshape(BPC * C * P, F),
            }
        )

    import os

    trace = bool(int(os.environ.get("KERNEL_TRACE", "0")))
    if trace:
        _register_ntff_hook()
    res = bass_utils.run_bass_kernel_spmd(
        nc, in_maps, core_ids=list(range(NCORES)), trace=trace
    )
    LAST_EXEC_NS = res.exec_time_ns
    _CACHE["last_res"] = res

    # Decode.  Packed hue cols are exact ints in f32: split via % and //.
    # Sign-sum cols decode as N_lt = (N - S)/2.  Sat counts come from the
    # PE PSUM accumulators (already plain cumulative counts).
    PK = int(PACK)
    hue_lo = np.zeros((2, 4))      # A,B,C,D
    hue_hi = np.zeros((2, 4))      # F,G,H,I tilde counts (u<0 side)
    E_sign = np.zeros(2)
    sign_sums = np.zeros((2, 9))
    C_sat = np.zeros((2, 9))
    for core_out in res.results:
        o = np.asarray(core_out["out"]).reshape(NITER, P, ACCW).astype(np.int64)
        s = np.asarray(core_out["sat"]).astype(np.int64)
        C_sat[0] += s[0:NSAT, 0]
        C_sat[1] += s[32 : 32 + NSAT, 1]
        for t, sl in ((0, slice(0, BPC)), (1, slice(BPC, NITER))):
            blk = o[sl]
            packed = blk[:, :, 0:4]
            hue_lo[t] += (packed % PK).sum(axis=(0, 1))
            hue_hi[t] += (packed // PK).sum(axis=(0, 1))
            # sign-sums live in the odd iterations of each half
            sign_sums[t] += blk[(1, 3), :, 10:19].sum(axis=(0, 1))
            E_sign[t] += blk[(1, 3), :, 4].sum() + blk[(1, 3), :, 5].sum()

    # hue cumulative counts [2, 9]
    E = (NPIX + E_sign) / 2.0
    C_hue = np.stack([
        hue_lo[:, 0], hue_lo[:, 1], hue_lo[:, 2], hue_lo[:, 3], E,
        NPIX - hue_hi[:, 0], NPIX - hue_hi[:, 1],
        NPIX - hue_hi[:, 2], NPIX - hue_hi[:, 3],
    ], axis=1)
    C_val = (NPIX - sign_sums) / 2.0

    loss = 0.0
    for wgt, Cc in ((ALPHA, C_hue), (BETA, C_sat), (GAMMA, C_val)):
        hist = np.zeros((2, 10))
        hist[:, 0] = Cc[:, 0]
        hist[:, 1:9] = Cc[:, 1:] - Cc[:, :-1]
        hist[:, 9] = NPIX - Cc[:, 8]
        loss += wgt * np.abs(hist[0] - hist[1]).mean()
    return np.asarray(loss, dtype=np.float32)


# revision 16
# speedup vs baseline: 1.2584x; 1.0547x over previous
"""ColorHistogramLoss Trainium2 kernel (8 NeuronCores, data-parallel).

Strategy: shard batch (32 -> 4 per core); each core streams its 25MB of
pixels through SBUF as 8 iterations of [128, 2048] plane-triples (4 real +
4 fake).  Counting is split across four engines so the DVE (the old
bottleneck) only carries what no other engine can:

- hue (9 edges): geometric ray tests on (u=g-b, v=b-r) as 4 dual-packed
  f32 custom-DVE ops (antipodal ray pairs share a boundary line; the
  sign(u) gate routes counts into a cnt + 4096*cnt packed accumulator)
  plus an E = #[u>=0] sign-sum on ScalarE.  f32 because the custom-DVE
  uop pipeline miscomputes on fp16 operands (measured).
- sat (9 edges): sat < c  <=>  mn/mx > 1-c.  ScalarE computes
  rmx = reciprocal(mx16); VectorE forms ratio16 = mn16 * rmx and then
  nine fp16 tensor_scalar indicator tiles (no accum -> 4x DVE mode,
  ~0.6us each); the TENSOR engine colsums each indicator via one-hot
  stationaries into a PSUM accumulator (216ns per 512-chunk, running
  concurrently with the DVE at no cost), accumulated over all 8
  iterations; one DVE reduce at the end reads the 18 totals.
- val (9 edges): Sign activations on fp16 mx pairs on ScalarE with fused
  accumulation (host decodes N_lt = (N - sign_sum)/2), batched over
  iteration pairs to amortize the fixed activation cost.
- min/max chains run in fp16 on the DVE at 2x (r16/g16/b16 casts ride
  the ScalarE Copy activation; Sign/Reciprocal/Copy share one act table
  so there is a single table load).

All counts are exact integers; only boundary-ulp pixels (fp16 rounding
of mx/mn/ratio) differ from the f32 reference (rel err ~6e-3 measured
against the reference loss, gate is 2e-2).
"""

import sys

if "/opt/trn_rl_repo" not in sys.path:
    sys.path.insert(0, "/opt/trn_rl_repo")

import numpy as np

from concourse import bacc, mybir, tile
from concourse import bass_utils

# ---- problem constants (hardcoded; kernel.py must be self-contained) ----
B, C, H, W = 32, 3, 512, 512
NCORES = 8
BPC = B // NCORES            # batches per core
P, F = 128, 2048             # SBUF tile: one [512,512] plane = [128, 2048]
NITER = 2 * BPC              # 4 real + 4 fake plane-triple iterations
ACCW = 20                    # accumulator columns per iteration
NPIX = B * H * W             # pixels per full histogram
ALPHA, BETA, GAMMA = 0.3, 0.4, 0.4

AF = mybir.AluOpType
F32 = mybir.dt.float32
F16 = mybir.dt.float16

LAST_EXEC_NS = None
_CACHE = {}

PACK = 4096.0  # dual-count packing: accum = cntA + PACK*cntB (exact in f32)
C23 = float(np.float32(2.0) / np.float32(3.0))

SVAL_EDGES = (0.1, 0.2, 0.3, 0.4, 0.5, 0.6, 0.7, 0.8, 0.9)
SAT_EDGES = (0.1, 0.2, 0.3, 0.4, 0.5, 0.6, 0.7, 0.8, 0.9)
NSAT = len(SAT_EDGES)
NK = NSAT + 1                 # PE psum rows: 9 sat edges + val-0.5


def _register_custom_ops():
    """Author + register fused DVE ops in the dve_ops registry at runtime
    (the repo list is read-only; registration is by-name so appending to the
    module-level OPS list is sufficient for table-gen and tracing)."""
    from concourse import dve_ops
    from concourse.dve_spec import (
        C0, C1, C2, Spec, Src0, Src1, Zero, One, _has_src1, lower, maxx,
        minn, select,
    )
    from concourse.dve_uop import DveOpSpec

    if hasattr(dve_ops, "HPA"):
        return dve_ops

    from operator import add as _add

    def _accref(body_fn):
        def ref(in0, in1, c0, c1, c2):
            b = body_fn(
                np.asarray(in0, np.float32),
                np.asarray(in1, np.float32) if in1 is not None else None,
                c0, c1, c2,
            ).astype(np.float32)
            return b, b.reshape(b.shape[0], -1).sum(axis=-1, keepdims=True)
        return ref

    # gate: 1 where u>=0 else PACK (routes the count into the high field)
    gate = select(Src0 >= Zero, One, C1)

    defs = [
        # hue pair, A-form: t = (Src0 + C0*Src1 <= 0); accum t*(1|C1 by sign)
        (
            "HPA",
            Spec(
                body=((Src0 + C0 * Src1) <= Zero) * gate,
                accum=_add,
                accum_init=Zero,
                reference=_accref(
                    lambda u, v, c0, c1, c2: ((u + np.float32(c0) * v) <= 0)
                    * np.where(u >= 0, 1.0, c1)
                ),
            ),
        ),
        # hue pair, B-form: t = (Src1 + C0*Src0 <= 0)
        (
            "HPB",
            Spec(
                body=((Src1 + C0 * Src0) <= Zero) * gate,
                accum=_add,
                accum_init=Zero,
                reference=_accref(
                    lambda u, v, c0, c1, c2: ((v + np.float32(c0) * u) <= 0)
                    * np.where(u >= 0, 1.0, c1)
                ),
            ),
        ),
    ]
    for name, spec in defs:
        row = 1 + len(dve_ops.OPS)
        shas = {}
        for ver in ("v3", "v4"):
            uops = lower(spec, ver=ver)
            shas[ver] = DveOpSpec(
                name=name, opcode=row, uops=uops, rd1_en=_has_src1(spec)
            ).sha(ver)
        op = dve_ops.DveOp(name, spec, False, uops_sha=shas)
        dve_ops.OPS.append(op)
        dve_ops.CUSTOM_DVE_SPECS[name] = spec
        dve_ops._SUB_OPCODE_FOR_NAME[name] = row
        setattr(dve_ops, name, op)
    return dve_ops


def _build():
    dve_ops = _register_custom_ops()
    nc = bacc.Bacc(
        "TRN2", target_bir_lowering=False, debug=False, num_devices=NCORES
    )
    xr = nc.dram_tensor("x_real", [BPC * C * P, F], F32, kind="ExternalInput").ap()
    xf = nc.dram_tensor("x_fake", [BPC * C * P, F], F32, kind="ExternalInput").ap()
    out = nc.dram_tensor("out", [NITER * P, ACCW], F32, kind="ExternalOutput").ap()
    sat_out = nc.dram_tensor("sat", [P, 2], F32, kind="ExternalOutput").ap()

    SIGN = mybir.ActivationFunctionType.Sign
    COPY = mybir.ActivationFunctionType.Copy
    LN = mybir.ActivationFunctionType.Ln

    with tile.TileContext(nc) as tc:
        with tc.tile_pool(name="main", bufs=2) as io_pool, tc.tile_pool(
            name="tmp", bufs=1
        ) as tmp_pool, tc.tile_pool(name="ps", bufs=1, space="PSUM") as ppool:
            # per-edge bias tiles for ScalarE Sign activations (bias = -edge)
            sbias = []
            for e in SVAL_EDGES + (0.0,):       # 0.0: E-count sign(u)
                bt = tmp_pool.tile([P, 1], F32, tag=f"sb{int(e*10)}",
                                   name=f"sb{int(e*10)}")
                nc.gpsimd.memset(bt[:], -e)
                sbias.append(bt)
            # one-hot stationaries for the PE colsum (col j = ones)
            stats = []
            for j in range(NK):
                st = tmp_pool.tile([P, NK], F16, tag=f"st{j}", name=f"st{j}")
                nc.vector.memset(st[:], 0.0)
                nc.vector.memset(st[:, j : j + 1], 1.0)
                stats.append(st)
            psum = ppool.tile([P, F], F32, tag="psum")
            scr2b = tmp_pool.tile([P, 2, F], F16, tag="scr2b", name="scr2b")
            ind_rot = [
                tmp_pool.tile([P, F], F16, tag=f"ind{k}", name=f"ind{k}")
                for k in range(2)
            ]
            acc_sat = tmp_pool.tile([P, 2], F32, tag="acc_sat", name="acc_sat")
            nc.vector.memset(acc_sat[:], 0.0)
            # f32 scratch shared by the custom-DVE dummy outs and the final
            # PSUM readback
            scr = tmp_pool.tile([P, F], F32, tag="scr", name="scr")

            V = nc.vector
            S = nc.scalar
            for it in range(NITER):
                src = xr if it < BPC else xf
                bi = it % BPC

                def plane(c):
                    q = bi * C + c
                    return src[q * P : (q + 1) * P, :]

                r = io_pool.tile([P, F], F32, tag="r")
                g = io_pool.tile([P, F], F32, tag="g")
                bl = io_pool.tile([P, F], F32, tag="bl")
                nc.sync.dma_start(g[:], plane(1))
                nc.sync.dma_start(bl[:], plane(2))
                nc.sync.dma_start(r[:], plane(0))

                # u and mx16 live in [P, 2, F] pair tiles so ScalarE can run
                # one batched activation over two iterations' data
                if it % 2 == 0:
                    upair = io_pool.tile([P, 2, F], F32, tag="upair")
                    mxpair = io_pool.tile([P, 2, F], F16, tag="mxpair")
                u = upair[:, it % 2, :]
                mx16 = mxpair[:, it % 2, :]
                v = tmp_pool.tile([P, F], F32, tag="v", name="v")
                r16 = io_pool.tile([P, F], F16, tag="r16")
                g16 = io_pool.tile([P, F], F16, tag="g16")
                b16 = io_pool.tile([P, F], F16, tag="b16")
                m1 = tmp_pool.tile([P, F], F16, tag="m1", name="m1")
                mn1 = tmp_pool.tile([P, F], F16, tag="mn1", name="mn1")
                mn16 = tmp_pool.tile([P, F], F16, tag="mn16", name="mn16")
                rmx = io_pool.tile([P, F], F16, tag="rmx")
                ratio = io_pool.tile([P, F], F16, tag="ratio")
                acc = io_pool.tile([P, ACCW], F32, tag="acc")

                # ScalarE: fp16 casts of the three planes
                S.activation(g16[:], g[:], COPY)
                S.activation(b16[:], bl[:], COPY)
                S.activation(r16[:], r[:], COPY)

                # DVE: u, v in f32 (custom-DVE hue ops need f32 operands)
                V.tensor_tensor(u, g[:], bl[:], AF.subtract)
                V.tensor_tensor(v[:], bl[:], r[:], AF.subtract)
                # hue pairs on (u, v): acc0..3
                V._custom_dve(dve_ops.HPA, out=scr[:], in0=u, in1=v[:],
                              s0=0.6, s1=PACK, accum_out=acc[:, 0:1])
                V._custom_dve(dve_ops.HPB, out=scr[:], in0=u, in1=v[:],
                              s0=0.8, s1=PACK, accum_out=acc[:, 1:2])
                V._custom_dve(dve_ops.HPB, out=scr[:], in0=u, in1=v[:],
                              s0=0.2, s1=PACK, accum_out=acc[:, 2:3])
                V._custom_dve(dve_ops.HPB, out=scr[:], in0=u, in1=v[:],
                              s0=-C23, s1=PACK, accum_out=acc[:, 3:4])
                # fp16 min/max chains (2x DVE mode)
                V.tensor_tensor(m1[:], r16[:], g16[:], AF.max)
                V.tensor_tensor(mx16, m1[:], b16[:], AF.max)
                V.tensor_tensor(mn1[:], r16[:], g16[:], AF.min)
                V.tensor_tensor(mn16[:], mn1[:], b16[:], AF.min)
                # sat < c  <=>  mn/mx > 1-c  <=>  ln(mn) - ln(mx) > ln(1-c).
                # (ScalarE Reciprocal is blocked for accuracy; Ln shares the
                # natural_log act table with Sign and Copy -> no reloads.
                # ln(0) = -inf keeps the mn==0 pixels in the right bin.)
                S.activation(rmx[:], mx16, LN)
                S.activation(ratio[:], mn16[:], LN)
                w = tmp_pool.tile([P, F], F16, tag="w", name="w")
                V.tensor_tensor(w[:], ratio[:], rmx[:], AF.subtract)
                # sat indicators (4x DVE mode, no accum) + PE colsum into PSUM
                base = 0 if it < BPC else 32
                rows = slice(base, base + NK)
                for j, c in enumerate(SAT_EDGES):
                    indt = ind_rot[j % 2][:]
                    V.tensor_scalar(indt, w[:], float(np.log(1.0 - c)), None,
                                    AF.is_gt)
                    for cj in range(4):
                        nc.tensor.matmul(
                            psum[rows, cj * 512 : (cj + 1) * 512],
                            stats[j][:, :],
                            indt[:, cj * 512 : (cj + 1) * 512],
                            start=(it % BPC == 0 and j == 0),
                            stop=False,
                        )

                # val-0.5 rides the PE path too (rebalances ScalarE -> PE)
                indt = ind_rot[NSAT % 2][:]
                V.tensor_scalar(indt, mx16, 0.5, None, AF.is_lt)
                for cj in range(4):
                    nc.tensor.matmul(
                        psum[rows, cj * 512 : (cj + 1) * 512],
                        stats[NSAT][:, :],
                        indt[:, cj * 512 : (cj + 1) * 512],
                        start=False,
                        stop=(it % BPC == BPC - 1 and cj == 3),
                    )

                if it % 2 == 1:
                    # ScalarE, batched over the iteration pair: val-9 + E
                    # acc10..18 (val sign-sums over 2*F on fp16 mx), acc4 (E)
                    for k in range(9):
                        if k == 4:      # 0.5 is counted on the PE path
                            continue
                        S.activation(
                            scr2b[:], mxpair[:], SIGN, bias=sbias[k][:],
                            accum_out=acc[:, 10 + k : 11 + k],
                        )
                    S.activation(
                        scr2b[:], upair[:], SIGN,
                        bias=sbias[9][:], accum_out=acc[:, 4:5],
                    )
                nc.sync.dma_start(out[it * P : (it + 1) * P, :], acc[:, :])

            # final: read the PE sat accumulators out of PSUM
            V.tensor_scalar(scr[0:NK, :], psum[0:NK, :], 1.0, None,
                            AF.mult, AF.add, accum_out=acc_sat[0:NK, 0:1])
            V.tensor_scalar(scr[0:NK, :], psum[32 : 32 + NK, :], 1.0,
                            None, AF.mult, AF.add,
                            accum_out=acc_sat[32 : 32 + NK, 1:2])
            nc.sync.dma_start(sat_out, acc_sat[:])

    nc.compile()
    return nc


def _register_ntff_hook():
    """Register the axon NTFF profiling hook (the container's antenv stub
    lacks axon_hooks, so trn_boot's registration was skipped). Also keep
    profile artifacts local instead of uploading to a share."""
    import types

    import antenv

    if "antenv.axon_hooks" not in sys.modules:
        mod = types.ModuleType("antenv.axon_hooks")
        holder = [None]
        mod.set_axon_ntff_profile_hook = lambda h: holder.__setitem__(0, h)
        mod.get_axon_ntff_profile_hook = lambda: holder[0]
        sys.modules["antenv.axon_hooks"] = mod
        antenv.axon_hooks = mod
    from antenv import axon_hooks

    if axon_hooks.get_axon_ntff_profile_hook() is None:
        from trn_agent_boot.trn_boot import _ntff_profile_via_ctypes

        axon_hooks.set_axon_ntff_profile_hook(
            _ntff_profile_via_ctypes("/opt/axon/libaxon_pjrt.so")
        )
    bass_utils.upload_artifacts = lambda tmpdir: tmpdir


def _get_nc():
    if "nc" not in _CACHE:
        _CACHE["nc"] = _build()
    return _CACHE["nc"]


def kernel(x_real: np.ndarray, x_fake: np.ndarray) -> np.ndarray:
    global LAST_EXEC_NS
    nc = _get_nc()

    in_maps = []
    for c in range(NCORES):
        sl = slice(c * BPC, (c + 1) * BPC)
        in_maps.append(
            {
                "x_real": np.ascontiguousarray(x_real[sl]).reshape(BPC * C * P, F),
                "x_fake": np.ascontiguousarray(x_fake[sl]).reshape(BPC * C * P, F),
            }
        )

    import os

    trace = bool(int(os.environ.get("KERNEL_TRACE", "0")))
    if trace:
        _register_ntff_hook()
    res = bass_utils.run_bass_kernel_spmd(
        nc, in_maps, core_ids=list(range(NCORES)), trace=trace
    )
    LAST_EXEC_NS = res.exec_time_ns
    _CACHE["last_res"] = res

    # Decode.  Packed hue cols are exact ints in f32: split via % and //.
    # Sign-sum cols decode as N_lt = (N - S)/2.  Sat counts come from the
    # PE PSUM accumulators (already plain cumulative counts).
    PK = int(PACK)
    hue_lo = np.zeros((2, 4))      # A,B,C,D
    hue_hi = np.zeros((2, 4))      # F,G,H,I tilde counts (u<0 side)
    E_sign = np.zeros(2)
    sign_sums = np.zeros((2, 9))
    C_sat = np.zeros((2, 9))
    C_val05 = np.zeros(2)
    for core_out in res.results:
        o = np.asarray(core_out["out"]).reshape(NITER, P, ACCW).astype(np.int64)
        s = np.asarray(core_out["sat"]).astype(np.int64)
        C_sat[0] += s[0:NSAT, 0]
        C_sat[1] += s[32 : 32 + NSAT, 1]
        C_val05[0] += s[NSAT, 0]
        C_val05[1] += s[32 + NSAT, 1]
        for t, sl in ((0, slice(0, BPC)), (1, slice(BPC, NITER))):
            blk = o[sl]
            packed = blk[:, :, 0:4]
            hue_lo[t] += (packed % PK).sum(axis=(0, 1))
            hue_hi[t] += (packed // PK).sum(axis=(0, 1))
            # sign-sums live in the odd iterations of each half
            sign_sums[t] += blk[(1, 3), :, 10:19].sum(axis=(0, 1))
            E_sign[t] += blk[(1, 3), :, 4].sum() + blk[(1, 3), :, 5].sum()

    # hue cumulative counts [2, 9]
    E = (NPIX + E_sign) / 2.0
    C_hue = np.stack([
        hue_lo[:, 0], hue_lo[:, 1], hue_lo[:, 2], hue_lo[:, 3], E,
        NPIX - hue_hi[:, 0], NPIX - hue_hi[:, 1],
        NPIX - hue_hi[:, 2], NPIX - hue_hi[:, 3],
    ], axis=1)
    C_val = (NPIX - sign_sums) / 2.0
    C_val[:, 4] = C_val05

    loss = 0.0
    for wgt, Cc in ((ALPHA, C_hue), (BETA, C_sat), (GAMMA, C_val)):
        hist = np.zeros((2, 10))
        hist[:, 0] = Cc[:, 0]
        hist[:, 1:9] = Cc[:, 1:] - Cc[:, :-1]
        hist[:, 9] = NPIX - Cc[:, 8]
        loss += wgt * np.abs(hist[0] - hist[1]).mean()
    return np.asarray(loss, dtype=np.float32)
